# revision 3
# baseline (speedup 1.0000x reference)
"""GCN layer (GCNConv + PReLU) on 8 Trainium2 NeuronCores.

Math: with deg[n] = in-degree(n)+1 and dinv = deg^-1/2, fold the edge
normalization dinv[src]*dinv[dst] into node scaling:

    h'[n]  = (x @ W)[n] * dinv[n]
    out[d] = dinv[d] * ( sum_{e: dst=d} h'[src_e] + h'[d] ) + b  -> PReLU

so per-edge work is a pure gather + scatter-add of h' rows.

Distribution (8 cores):
  Launch 1: row-shard x (6250 rows/core); each core computes its h'.T shard
    [128, 6272] via TensorE (W stationary), scaled by dinv on VectorE.
  Host: concatenates/transposes shards into the row-major gather table
    hD [50176, 128] (the halo exchange).
  Launch 2: dst-shard the aggregation. Each core owns 6272 padded dst rows =
    13 groups of 512 (last 128). Per group: batch-gather h' rows of all
    in-edges (gpsimd dma_gather, int16 idxs => split src < / >= 32768 with a
    rebased table pointer), then scatter-add within the group by one-hot
    selection-matrix matmuls accumulating into a PSUM tile out.T [128h, 512d]
    (TensorE contracts over the 128-edge chunk dim; duplicate dsts
    accumulate). Epilogue on VectorE: + self-loop h'.T, * dinv, + bias,
    PReLU. Output is out.T per core; host re-transposes.

The Q7 descriptor-generation loop of dma_gather (~8.4 ns/row) is the
bottleneck; PE/DVE/SDMA work is hidden under it, so everything runs in exact
fp32 (F32R=True switches the scatter matmuls to the faster rounded-fp32 PE
path if PE ever binds).
"""
import sys
import numpy as np

try:
    import concourse.bacc as bacc
except ImportError:  # toolchain lives in the trn repo
    sys.path.insert(0, "/opt/trn_rl_repo")
    import concourse.bacc as bacc

import concourse.bass as bass
import concourse.mybir as mybir
import concourse.tile as tile
from concourse.bass_utils import run_bass_kernel_spmd

F32 = mybir.dt.float32
F32R = mybir.dt.float32r
I16 = mybir.dt.int16

N = 50000
IN_DIM = 512
HID = 128
NCORES = 8
NSH = N // NCORES            # 6250 nodes per core
PAD = 6272                   # padded shard rows (49 * 128)
HD_ROWS = NCORES * PAD       # 50176 gather-table rows (padded ids)
SPLIT = 32768                # int16 gather split point (padded ids)
NG = 13                      # dst groups per core: 12 x 512 + 1 x 128
GW = 512                     # group width
GW_LAST = 128
KCH = IN_DIM // 128          # 4 contraction chunks
SENT = 4096.0                # dst sentinel (outside any iota)

USE_F32R = False             # rounded-fp32 PE path for scatter matmuls

last_exec_ns = []
_nc_cache = {}


def _build_phase1():
    nc = bacc.Bacc("TRN2", target_bir_lowering=False, debug=False,
                   num_devices=NCORES)
    xT = nc.dram_tensor("xT", [IN_DIM, PAD], F32, kind="ExternalInput").ap()
    Wd = nc.dram_tensor("W", [IN_DIM, HID], F32, kind="ExternalInput").ap()
    dvr = nc.dram_tensor("dinvrep", [128, PAD], F32, kind="ExternalInput").ap()
    hsT = nc.dram_tensor("hshT", [128, PAD], F32, kind="ExternalOutput").ap()

    with tile.TileContext(nc) as tc:
        with (
            tc.tile_pool(name="const", bufs=1) as cpool,
            tc.tile_pool(name="work", bufs=6) as wpool,
            tc.tile_pool(name="psum", bufs=4, space="PSUM") as ppool,
        ):
            Wt = cpool.tile([128, KCH, HID], F32, name="Wt")
            dvt = cpool.tile([128, PAD], F32, name="dvt")
            nc.sync.dma_start(out=Wt[:], in_=Wd.rearrange("(k p) h -> p k h", p=128))
            nc.sync.dma_start(out=dvt[:], in_=dvr[:])
            for g in range(NG):
                w = GW if g < NG - 1 else GW_LAST
                c0 = g * GW
                ps = ppool.tile([128, w], F32, name=f"ps{g}", tag="ps",
                                space="PSUM", padded_shape=[128, GW])
                for k in range(KCH):
                    xk = wpool.tile([128, w], F32, name=f"x{g}_{k}", tag="xk",
                                    padded_shape=[128, GW])
                    nc.sync.dma_start(
                        out=xk[:],
                        in_=xT[k * 128:(k + 1) * 128, c0:c0 + w])
                    nc.tensor.matmul(out=ps[:], lhsT=Wt[:, k, :], rhs=xk[:],
                                     start=(k == 0), stop=(k == KCH - 1))
                hT = wpool.tile([128, w], F32, name=f"h{g}", tag="hT",
                                padded_shape=[128, GW])
                nc.vector.tensor_tensor(out=hT[:], in0=ps[:],
                                        in1=dvt[:, c0:c0 + w],
                                        op=mybir.AluOpType.mult)
                nc.sync.dma_start(out=hsT[:, c0:c0 + w], in_=hT[:])
    nc.compile()
    return nc


def _build_phase2(caps):
    """caps: list of NG tuples (NL, NH) chunk capacities per group."""
    totcol = sum(16 * (nl + nh) for nl, nh in caps)   # idx cols ( (nl+nh)*128/16 * 2 ... per group: (nl+nh)*8*2? )
    # idx cols per group: (nl + nh) * 128 / 16 = (nl + nh) * 8
    totcol = sum(8 * (nl + nh) for nl, nh in caps)
    totch = sum(nl + nh for nl, nh in caps)

    nc = bacc.Bacc("TRN2", target_bir_lowering=False, debug=False,
                   num_devices=NCORES)
    hD = nc.dram_tensor("hD", [HD_ROWS, HID], F32, kind="ExternalInput").ap()
    hTs = nc.dram_tensor("hshT", [128, PAD], F32, kind="ExternalInput").ap()
    ixd = nc.dram_tensor("idx", [128, totcol], I16, kind="ExternalInput").ap()
    dsd = nc.dram_tensor("dstv", [128, totch], F32, kind="ExternalInput").ap()
    dvr = nc.dram_tensor("dinvrep", [128, PAD], F32, kind="ExternalInput").ap()
    iod = nc.dram_tensor("iota", [128, GW], F32, kind="ExternalInput").ap()
    bid = nc.dram_tensor("bias", [128, 1], F32, kind="ExternalInput").ap()
    pwd = nc.dram_tensor("prelu", [128, 1], F32, kind="ExternalInput").ap()
    od = nc.dram_tensor("outT", [128, PAD], F32, kind="ExternalOutput").ap()

    gdt = F32R if USE_F32R else F32
    with tile.TileContext(nc) as tc:
        with (
            tc.tile_pool(name="const", bufs=1) as cpool,
            tc.tile_pool(name="gp", bufs=2) as gpool,
            tc.tile_pool(name="work", bufs=4) as wpool,
            tc.tile_pool(name="ep", bufs=2) as epool,
            tc.tile_pool(name="psum", bufs=2, space="PSUM") as ppool,
        ):
            ix_t = cpool.tile([128, totcol], I16, name="ix_t")
            ds_t = cpool.tile([128, totch], F32, name="ds_t")
            io_t = cpool.tile([128, GW], F32, name="io_t")
            bi_t = cpool.tile([128, 1], F32, name="bi_t")
            pw_t = cpool.tile([128, 1], F32, name="pw_t")
            nc.sync.dma_start(out=ix_t[:], in_=ixd[:])
            nc.sync.dma_start(out=ds_t[:], in_=dsd[:])
            nc.sync.dma_start(out=io_t[:], in_=iod[:])
            nc.sync.dma_start(out=bi_t[:], in_=bid[:])
            nc.sync.dma_start(out=pw_t[:], in_=pwd[:])

            coff = 0   # idx col offset
            choff = 0  # chunk offset
            NL0 = max(nl for nl, nh in caps)
            NH0 = max(nh for nl, nh in caps)
            for g in range(NG):
                nl, nh = caps[g]
                w = GW if g < NG - 1 else GW_LAST
                c0 = g * GW
                GL = gpool.tile([128, nl, HID], gdt, name=f"GL{g}", tag="GL",
                                padded_shape=[128, NL0, HID])
                GH = gpool.tile([128, nh, HID], gdt, name=f"GH{g}", tag="GH",
                                padded_shape=[128, NH0, HID])
                nc.gpsimd.dma_gather(
                    out_ap=GL[:], in_ap=hD[0:SPLIT, :].bitcast(gdt),
                    idxs_ap=ix_t[:, coff:coff + nl * 8],
                    num_idxs=nl * 128, num_idxs_reg=nl * 128,
                    elem_size=HID, single_packet=False)
                nc.gpsimd.dma_gather(
                    out_ap=GH[:], in_ap=hD[SPLIT:HD_ROWS, :].bitcast(gdt),
                    idxs_ap=ix_t[:, coff + nl * 8:coff + (nl + nh) * 8],
                    num_idxs=nh * 128, num_idxs_reg=nh * 128,
                    elem_size=HID, single_packet=False)
                coff += (nl + nh) * 8

                ps = ppool.tile([128, w], F32, name=f"ps{g}", tag="ps",
                                space="PSUM", padded_shape=[128, GW])
                for c in range(nl + nh):
                    S = wpool.tile([128, w], gdt, name=f"S{g}_{c}", tag="S",
                                   padded_shape=[128, GW])
                    nc.vector.tensor_scalar(
                        out=S[:], in0=io_t[:, :w],
                        scalar1=ds_t[:, choff + c:choff + c + 1], scalar2=None,
                        op0=mybir.AluOpType.is_equal)
                    gsl = (GL[:, c, :] if c < nl else GH[:, c - nl, :])
                    nc.tensor.matmul(out=ps[:], lhsT=gsl, rhs=S[:],
                                     start=(c == 0), stop=(c == nl + nh - 1))
                choff += nl + nh

                # epilogue in out.T layout [128 h, w d]
                sl = epool.tile([128, w], F32, name=f"sl{g}", tag="sl",
                                padded_shape=[128, GW])
                nc.sync.dma_start(out=sl[:], in_=hTs[:, c0:c0 + w])
                dv = epool.tile([128, w], F32, name=f"dv{g}", tag="dv",
                                padded_shape=[128, GW])
                nc.sync.dma_start(out=dv[:], in_=dvr[:, c0:c0 + w])
                y0 = epool.tile([128, w], F32, name=f"y0_{g}", tag="y0",
                                padded_shape=[128, GW])
                nc.vector.tensor_tensor(out=y0[:], in0=ps[:], in1=sl[:],
                                        op=mybir.AluOpType.add)
                y1 = epool.tile([128, w], F32, name=f"y1_{g}", tag="y1",
                                padded_shape=[128, GW])
                nc.vector.tensor_tensor(out=y1[:], in0=y0[:], in1=dv[:],
                                        op=mybir.AluOpType.mult)
                y2 = epool.tile([128, w], F32, name=f"y2_{g}", tag="y2",
                                padded_shape=[128, GW])
                nc.vector.tensor_scalar(out=y2[:], in0=y1[:],
                                        scalar1=bi_t[:, 0:1], scalar2=None,
                                        op0=mybir.AluOpType.add)
                pos = epool.tile([128, w], F32, name=f"pp{g}", tag="pp",
                                 padded_shape=[128, GW])
                nc.vector.tensor_scalar_max(pos[:], y2[:], 0.0)
                neg = epool.tile([128, w], F32, name=f"nn{g}", tag="nn",
                                 padded_shape=[128, GW])
                nc.vector.tensor_scalar_min(neg[:], y2[:], 0.0)
                ng2 = epool.tile([128, w], F32, name=f"n2{g}", tag="n2",
                                 padded_shape=[128, GW])
                nc.vector.tensor_scalar(out=ng2[:], in0=neg[:],
                                        scalar1=pw_t[:, 0:1], scalar2=None,
                                        op0=mybir.AluOpType.mult)
                yo = epool.tile([128, w], F32, name=f"yo{g}", tag="yo",
                                padded_shape=[128, GW])
                nc.vector.tensor_tensor(out=yo[:], in0=pos[:], in1=ng2[:],
                                        op=mybir.AluOpType.add)
                nc.sync.dma_start(out=od[:, c0:c0 + w], in_=yo[:])
    nc.compile()
    return nc


def _pack_core(spid, dloc, group, caps):
    """Pack one core's edges into idx [128, totcol] i16 and dstv [128, totch]
    f32 arrays. spid: padded src ids; dloc: dst offset within group;
    group: group id per edge."""
    totcol = sum(8 * (nl + nh) for nl, nh in caps)
    totch = sum(nl + nh for nl, nh in caps)
    idx16 = np.zeros((16, totcol), dtype=np.int16)
    dstv = np.full((128, totch), SENT, dtype=np.float32)
    coff = 0
    choff = 0
    hi_all = spid >= SPLIT
    for g in range(NG):
        nl, nh = caps[g]
        in_g = group == g
        s_g = spid[in_g]
        d_g = dloc[in_g]
        hi = s_g >= SPLIT
        for (mask, ncap, base) in ((~hi, nl, 0), (hi, nh, SPLIT)):
            s = s_g[mask] - base
            d = d_g[mask]
            assert len(s) <= ncap * 128, (g, len(s), ncap)
            v = np.zeros(ncap * 128, dtype=np.int16)
            v[:len(s)] = s.astype(np.int16)
            idx16[:, coff:coff + ncap * 8] = v.reshape(ncap * 8, 16).T
            coff += ncap * 8
            dd = np.full(ncap * 128, SENT, dtype=np.float32)
            dd[:len(d)] = d.astype(np.float32)
            dstv[:, choff:choff + ncap] = dd.reshape(ncap, 128).T
            choff += ncap
    return np.tile(idx16, (8, 1)), dstv


def kernel(x, edge_index, W, b, prelu_w):
    global last_exec_ns
    last_exec_ns = []
    x = np.asarray(x, dtype=np.float32)
    edge_index = np.asarray(edge_index, dtype=np.int32)
    W = np.asarray(W, dtype=np.float32)
    b = np.asarray(b, dtype=np.float32)
    prelu_w = np.asarray(prelu_w, dtype=np.float32)

    src = edge_index[0].astype(np.int64)
    dst = edge_index[1].astype(np.int64)

    deg = (np.bincount(dst, minlength=N) + 1).astype(np.float32)
    dinv = (1.0 / np.sqrt(deg)).astype(np.float32)

    # padded node ids: core-shards of 6272 rows
    core = dst // NSH
    spid = (src // NSH) * PAD + (src % NSH)
    dl_all = dst % NSH
    group = dl_all // GW
    dloc = dl_all - group * GW

    # per (core, group, L/H) counts -> per-group global capacities
    hi = spid >= SPLIT
    key = (core * NG + group) * 2 + hi
    cnt = np.bincount(key, minlength=NCORES * NG * 2).reshape(NCORES, NG, 2)
    caps = []
    for g in range(NG):
        nl = int(np.ceil(cnt[:, g, 0].max() / 128))
        nh = int(np.ceil(cnt[:, g, 1].max() / 128))
        caps.append((max(nl, 1), max(nh, 1)))
    caps = tuple(caps)

    dinv_pad = np.zeros((NCORES, PAD), dtype=np.float32)
    dinv_pad[:, :NSH] = dinv.reshape(NCORES, NSH)
    dinvrep = [np.tile(d.reshape(1, PAD), (128, 1)) for d in dinv_pad]

    # ---- launch 1 ----
    if "p1" not in _nc_cache:
        _nc_cache["p1"] = _build_phase1()
    in1 = []
    for c in range(NCORES):
        xs = np.zeros((IN_DIM, PAD), dtype=np.float32)
        xs[:, :NSH] = x[c * NSH:(c + 1) * NSH, :].T
        in1.append({"xT": xs, "W": W, "dinvrep": dinvrep[c]})
    r1 = run_bass_kernel_spmd(_nc_cache["p1"], in1,
                              core_ids=list(range(NCORES)))
    last_exec_ns.append(r1.exec_time_ns)
    hshT = [r1.results[c]["hshT"] for c in range(NCORES)]    # [128, PAD] each

    hD = np.concatenate([t.T for t in hshT], axis=0)         # [50176, 128]
    hD = np.ascontiguousarray(hD)

    # ---- launch 2 ----
    ckey = ("p2", caps, USE_F32R)
    if ckey not in _nc_cache:
        _nc_cache[ckey] = _build_phase2(caps)
    iota_np = np.tile(np.arange(GW, dtype=np.float32), (128, 1))
    bias_np = b.reshape(128, 1).astype(np.float32)
    prw_np = prelu_w.reshape(128, 1).astype(np.float32)

    in2 = []
    for c in range(NCORES):
        sel = core == c
        idx16, dstv = _pack_core(spid[sel], dloc[sel], group[sel], caps)
        in2.append({"hD": hD, "hshT": hshT[c], "idx": idx16, "dstv": dstv,
                    "dinvrep": dinvrep[c], "iota": iota_np, "bias": bias_np,
                    "prelu": prw_np})
    r2 = run_bass_kernel_spmd(_nc_cache[ckey], in2,
                              core_ids=list(range(NCORES)))
    last_exec_ns.append(r2.exec_time_ns)

    out = np.empty((N, HID), dtype=np.float32)
    for c in range(NCORES):
        out[c * NSH:(c + 1) * NSH] = r2.results[c]["outT"][:, :NSH].T
    return out


# revision 4
# speedup vs baseline: 1.3124x; 1.3124x over previous
"""GCN layer (GCNConv + PReLU) on 8 Trainium2 NeuronCores.

Math: with deg[n] = in-degree(n)+1 and dinv = deg^-1/2, fold the edge
normalization dinv[src]*dinv[dst] into node scaling:

    h'[n]  = (x @ W)[n] * dinv[n]
    out[d] = dinv[d] * ( sum_{e: dst=d} h'[src_e] + h'[d] ) + b  -> PReLU

so per-edge work is a pure gather + scatter-add of h' rows.

Distribution (8 cores):
  Launch 1: row-shard x (6250 rows/core); each core computes its h'.T shard
    [128, 6272] via TensorE (W stationary), scaled by dinv on VectorE.
  Host: concatenates/transposes shards into the row-major gather table
    hD [50176, 128] (the halo exchange).
  Launch 2: dst-shard the aggregation. Each core owns 6272 padded dst rows =
    13 groups of 512 (last 128). Per group: batch-gather h' rows of all
    in-edges (gpsimd dma_gather, int16 idxs => split src < / >= 32768 with a
    rebased table pointer), then scatter-add within the group by one-hot
    selection-matrix matmuls accumulating into a PSUM tile out.T [128h, 512d]
    (TensorE contracts over the 128-edge chunk dim; duplicate dsts
    accumulate). Epilogue on VectorE: + self-loop h'.T, * dinv, + bias,
    PReLU. Output is out.T per core; host re-transposes.

The Q7 descriptor-generation loop of dma_gather (~8.4 ns/row) is the
bottleneck; PE/DVE/SDMA work is hidden under it, so everything runs in exact
fp32 (F32R=True switches the scatter matmuls to the faster rounded-fp32 PE
path if PE ever binds).
"""
import sys
import numpy as np

try:
    import concourse.bacc as bacc
except ImportError:  # toolchain lives in the trn repo
    sys.path.insert(0, "/opt/trn_rl_repo")
    import concourse.bacc as bacc

import concourse.bass as bass
import concourse.mybir as mybir
import concourse.tile as tile
from concourse.bass_utils import run_bass_kernel_spmd

F32 = mybir.dt.float32
F32R = mybir.dt.float32r
I16 = mybir.dt.int16

N = 50000
IN_DIM = 512
HID = 128
NCORES = 8
NSH = N // NCORES            # 6250 nodes per core
PAD = 6272                   # padded shard rows (49 * 128)
HD_ROWS = NCORES * PAD       # 50176 gather-table rows (padded ids)
SPLIT = 32768                # int16 gather split point (padded ids)
NG = 13                      # dst groups per core: 12 x 512 + 1 x 128
GW = 512                     # group width
GW_LAST = 128
KCH = IN_DIM // 128          # 4 contraction chunks
SENT = 4096.0                # dst sentinel (outside any iota)

USE_F32R = False             # rounded-fp32 PE path for scatter matmuls

last_exec_ns = []
_nc_cache = {}


def _build_phase1():
    nc = bacc.Bacc("TRN2", target_bir_lowering=False, debug=False,
                   num_devices=NCORES)
    xT = nc.dram_tensor("xT", [IN_DIM, PAD], F32, kind="ExternalInput").ap()
    Wd = nc.dram_tensor("W", [IN_DIM, HID], F32, kind="ExternalInput").ap()
    dvr = nc.dram_tensor("dinvrep", [128, PAD], F32, kind="ExternalInput").ap()
    hsT = nc.dram_tensor("hshT", [128, PAD], F32, kind="ExternalOutput").ap()

    with tile.TileContext(nc) as tc:
        with (
            tc.tile_pool(name="const", bufs=1) as cpool,
            tc.tile_pool(name="work", bufs=6) as wpool,
            tc.tile_pool(name="psum", bufs=4, space="PSUM") as ppool,
        ):
            Wt = cpool.tile([128, KCH, HID], F32, name="Wt")
            dvt = cpool.tile([128, PAD], F32, name="dvt")
            nc.sync.dma_start(out=Wt[:], in_=Wd.rearrange("(k p) h -> p k h", p=128))
            nc.sync.dma_start(out=dvt[:], in_=dvr[:])
            for g in range(NG):
                w = GW if g < NG - 1 else GW_LAST
                c0 = g * GW
                ps = ppool.tile([128, w], F32, name=f"ps{g}", tag="ps",
                                space="PSUM", padded_shape=[128, GW])
                for k in range(KCH):
                    xk = wpool.tile([128, w], F32, name=f"x{g}_{k}", tag="xk",
                                    padded_shape=[128, GW])
                    nc.sync.dma_start(
                        out=xk[:],
                        in_=xT[k * 128:(k + 1) * 128, c0:c0 + w])
                    nc.tensor.matmul(out=ps[:], lhsT=Wt[:, k, :], rhs=xk[:],
                                     start=(k == 0), stop=(k == KCH - 1))
                hT = wpool.tile([128, w], F32, name=f"h{g}", tag="hT",
                                padded_shape=[128, GW])
                nc.vector.tensor_tensor(out=hT[:], in0=ps[:],
                                        in1=dvt[:, c0:c0 + w],
                                        op=mybir.AluOpType.mult)
                nc.sync.dma_start(out=hsT[:, c0:c0 + w], in_=hT[:])
    nc.compile()
    return nc


def _build_phase2(NL, NH):
    """NL/NH: global per-128-dst-block chunk capacities for the L/H gather
    groups. Gathers batch 4 blocks (one 512-dst group) per call; one-hot
    scatter matmuls run per block (N=128) into column slices of the group's
    PSUM accumulator out.T [128h, 512d]."""
    NB = PAD // 128                 # 49 blocks/core
    totcol = NB * 8 * (NL + NH)     # idx cols
    totch = NB * (NL + NH)          # dstv cols

    nc = bacc.Bacc("TRN2", target_bir_lowering=False, debug=False,
                   num_devices=NCORES)
    hD = nc.dram_tensor("hD", [HD_ROWS, HID], F32, kind="ExternalInput").ap()
    hTs = nc.dram_tensor("hshT", [128, PAD], F32, kind="ExternalInput").ap()
    ixd = nc.dram_tensor("idx", [128, totcol], I16, kind="ExternalInput").ap()
    dsd = nc.dram_tensor("dstv", [128, totch], F32, kind="ExternalInput").ap()
    dvr = nc.dram_tensor("dinvrep", [128, PAD], F32, kind="ExternalInput").ap()
    iod = nc.dram_tensor("iota", [128, 128], F32, kind="ExternalInput").ap()
    bid = nc.dram_tensor("bias", [128, 1], F32, kind="ExternalInput").ap()
    pwd = nc.dram_tensor("prelu", [128, 1], F32, kind="ExternalInput").ap()
    od = nc.dram_tensor("outT", [128, PAD], F32, kind="ExternalOutput").ap()

    gdt = F32R if USE_F32R else F32
    with tile.TileContext(nc) as tc:
        with (
            tc.tile_pool(name="const", bufs=1) as cpool,
            tc.tile_pool(name="gp", bufs=2) as gpool,
            tc.tile_pool(name="work", bufs=6) as wpool,
            tc.tile_pool(name="ep", bufs=2) as epool,
            tc.tile_pool(name="psum", bufs=2, space="PSUM") as ppool,
        ):
            ix_t = cpool.tile([128, totcol], I16, name="ix_t")
            ds_t = cpool.tile([128, totch], F32, name="ds_t")
            io_t = cpool.tile([128, 128], F32, name="io_t")
            bi_t = cpool.tile([128, 1], F32, name="bi_t")
            pw_t = cpool.tile([128, 1], F32, name="pw_t")
            nc.sync.dma_start(out=ix_t[:], in_=ixd[:])
            nc.sync.dma_start(out=ds_t[:], in_=dsd[:])
            nc.sync.dma_start(out=io_t[:], in_=iod[:])
            nc.sync.dma_start(out=bi_t[:], in_=bid[:])
            nc.sync.dma_start(out=pw_t[:], in_=pwd[:])

            for g in range(NG):
                nb = 4 if g < NG - 1 else 1   # blocks in this group
                w = nb * 128
                c0 = g * GW
                b0 = g * 4                    # first block id
                GL = gpool.tile([128, nb * NL, HID], gdt, name=f"GL{g}",
                                tag="GL", padded_shape=[128, 4 * NL, HID])
                GH = gpool.tile([128, nb * NH, HID], gdt, name=f"GH{g}",
                                tag="GH", padded_shape=[128, 4 * NH, HID])
                coff = b0 * 8 * (NL + NH)
                nc.gpsimd.dma_gather(
                    out_ap=GL[:], in_ap=hD[0:SPLIT, :].bitcast(gdt),
                    idxs_ap=ix_t[:, coff:coff + nb * NL * 8],
                    num_idxs=nb * NL * 128, num_idxs_reg=nb * NL * 128,
                    elem_size=HID, single_packet=False)
                nc.gpsimd.dma_gather(
                    out_ap=GH[:], in_ap=hD[SPLIT:HD_ROWS, :].bitcast(gdt),
                    idxs_ap=ix_t[:, coff + nb * NL * 8:coff + nb * (NL + NH) * 8],
                    num_idxs=nb * NH * 128, num_idxs_reg=nb * NH * 128,
                    elem_size=HID, single_packet=False)

                ps = ppool.tile([128, w], F32, name=f"ps{g}", tag="ps",
                                space="PSUM", padded_shape=[128, GW])
                for bi in range(nb):
                    choff = (b0 + bi) * (NL + NH)
                    for c in range(NL + NH):
                        S = wpool.tile([128, 128], gdt, name=f"S{g}_{bi}_{c}",
                                       tag="S")
                        nc.vector.tensor_scalar(
                            out=S[:], in0=io_t[:],
                            scalar1=ds_t[:, choff + c:choff + c + 1],
                            scalar2=None, op0=mybir.AluOpType.is_equal)
                        gsl = (GL[:, bi * NL + c, :] if c < NL
                               else GH[:, bi * NH + (c - NL), :])
                        nc.tensor.matmul(
                            out=ps[:, bi * 128:(bi + 1) * 128],
                            lhsT=gsl, rhs=S[:],
                            start=(c == 0), stop=(c == NL + NH - 1))

                # epilogue in out.T layout [128 h, w d]
                sl = epool.tile([128, w], F32, name=f"sl{g}", tag="sl",
                                padded_shape=[128, GW])
                nc.sync.dma_start(out=sl[:], in_=hTs[:, c0:c0 + w])
                dv = epool.tile([128, w], F32, name=f"dv{g}", tag="dv",
                                padded_shape=[128, GW])
                nc.sync.dma_start(out=dv[:], in_=dvr[:, c0:c0 + w])
                y0 = epool.tile([128, w], F32, name=f"y0_{g}", tag="y0",
                                padded_shape=[128, GW])
                nc.vector.tensor_tensor(out=y0[:], in0=ps[:], in1=sl[:],
                                        op=mybir.AluOpType.add)
                y1 = epool.tile([128, w], F32, name=f"y1_{g}", tag="y1",
                                padded_shape=[128, GW])
                nc.vector.tensor_tensor(out=y1[:], in0=y0[:], in1=dv[:],
                                        op=mybir.AluOpType.mult)
                pos = epool.tile([128, w], F32, name=f"pp{g}", tag="pp",
                                 padded_shape=[128, GW])
                nc.vector.tensor_scalar(out=pos[:], in0=y1[:],
                                        scalar1=bi_t[:, 0:1], scalar2=0.0,
                                        op0=mybir.AluOpType.add,
                                        op1=mybir.AluOpType.max)
                neg = epool.tile([128, w], F32, name=f"nn{g}", tag="nn",
                                 padded_shape=[128, GW])
                nc.vector.tensor_scalar(out=neg[:], in0=y1[:],
                                        scalar1=bi_t[:, 0:1], scalar2=0.0,
                                        op0=mybir.AluOpType.add,
                                        op1=mybir.AluOpType.min)
                ng2 = epool.tile([128, w], F32, name=f"n2{g}", tag="n2",
                                 padded_shape=[128, GW])
                nc.vector.tensor_scalar(out=ng2[:], in0=neg[:],
                                        scalar1=pw_t[:, 0:1], scalar2=None,
                                        op0=mybir.AluOpType.mult)
                yo = epool.tile([128, w], F32, name=f"yo{g}", tag="yo",
                                padded_shape=[128, GW])
                nc.vector.tensor_tensor(out=yo[:], in0=pos[:], in1=ng2[:],
                                        op=mybir.AluOpType.add)
                nc.sync.dma_start(out=od[:, c0:c0 + w], in_=yo[:])
    nc.compile()
    return nc


def _pack_core(spid, bloc, dloc, NL, NH):
    """Pack one core's edges into idx [128, totcol] i16 and dstv [128, totch]
    f32. spid: padded src ids; bloc: 128-dst block id (0..48); dloc: dst
    offset within block (0..127)."""
    NB = PAD // 128
    idx16 = np.zeros((16, NB * 8 * (NL + NH)), dtype=np.int16)
    dstv = np.full((128, NB * (NL + NH)), SENT, dtype=np.float32)
    for b in range(NB):
        in_b = bloc == b
        s_b = spid[in_b]
        d_b = dloc[in_b]
        hi = s_b >= SPLIT
        g, bi = divmod(b, 4)
        base_col = (g * 4) * 8 * (NL + NH)
        nb = 4 if g < NG - 1 else 1
        for (mask, ncap, base, slot0) in (
            (~hi, NL, 0, base_col + bi * NL * 8),
            (hi, NH, SPLIT, base_col + nb * NL * 8 + bi * NH * 8),
        ):
            sv = s_b[mask] - base
            dd = d_b[mask]
            assert len(sv) <= ncap * 128, (b, len(sv), ncap)
            v = np.zeros(ncap * 128, dtype=np.int16)
            v[:len(sv)] = sv.astype(np.int16)
            idx16[:, slot0:slot0 + ncap * 8] = v.reshape(ncap * 8, 16).T
        choff = b * (NL + NH)
        dL = np.full(NL * 128, SENT, dtype=np.float32)
        dH = np.full(NH * 128, SENT, dtype=np.float32)
        dL[:np.count_nonzero(~hi)] = d_b[~hi].astype(np.float32)
        dH[:np.count_nonzero(hi)] = d_b[hi].astype(np.float32)
        dstv[:, choff:choff + NL] = dL.reshape(NL, 128).T
        dstv[:, choff + NL:choff + NL + NH] = dH.reshape(NH, 128).T
    return np.tile(idx16, (8, 1)), dstv


def kernel(x, edge_index, W, b, prelu_w):
    global last_exec_ns
    last_exec_ns = []
    x = np.asarray(x, dtype=np.float32)
    edge_index = np.asarray(edge_index, dtype=np.int32)
    W = np.asarray(W, dtype=np.float32)
    b = np.asarray(b, dtype=np.float32)
    prelu_w = np.asarray(prelu_w, dtype=np.float32)

    src = edge_index[0].astype(np.int64)
    dst = edge_index[1].astype(np.int64)

    deg = (np.bincount(dst, minlength=N) + 1).astype(np.float32)
    dinv = (1.0 / np.sqrt(deg)).astype(np.float32)

    # padded node ids: core-shards of 6272 rows
    core = dst // NSH
    spid = (src // NSH) * PAD + (src % NSH)
    dl_all = dst % NSH
    bloc = dl_all // 128
    dloc = dl_all - bloc * 128

    # per (core, block, L/H) counts -> global capacities
    NB = PAD // 128
    hi = spid >= SPLIT
    key = (core * NB + bloc) * 2 + hi
    cnt = np.bincount(key, minlength=NCORES * NB * 2).reshape(-1, 2)
    NL = max(1, int(np.ceil(cnt[:, 0].max() / 128)))
    NH = max(1, int(np.ceil(cnt[:, 1].max() / 128)))

    dinv_pad = np.zeros((NCORES, PAD), dtype=np.float32)
    dinv_pad[:, :NSH] = dinv.reshape(NCORES, NSH)
    dinvrep = [np.tile(d.reshape(1, PAD), (128, 1)) for d in dinv_pad]

    # ---- launch 1 ----
    if "p1" not in _nc_cache:
        _nc_cache["p1"] = _build_phase1()
    in1 = []
    for c in range(NCORES):
        xs = np.zeros((IN_DIM, PAD), dtype=np.float32)
        xs[:, :NSH] = x[c * NSH:(c + 1) * NSH, :].T
        in1.append({"xT": xs, "W": W, "dinvrep": dinvrep[c]})
    r1 = run_bass_kernel_spmd(_nc_cache["p1"], in1,
                              core_ids=list(range(NCORES)))
    last_exec_ns.append(r1.exec_time_ns)
    hshT = [r1.results[c]["hshT"] for c in range(NCORES)]    # [128, PAD] each

    hD = np.concatenate([t.T for t in hshT], axis=0)         # [50176, 128]
    hD = np.ascontiguousarray(hD)

    # ---- launch 2 ----
    ckey = ("p2", NL, NH, USE_F32R)
    if ckey not in _nc_cache:
        _nc_cache[ckey] = _build_phase2(NL, NH)
    iota_np = np.tile(np.arange(128, dtype=np.float32), (128, 1))
    bias_np = b.reshape(128, 1).astype(np.float32)
    prw_np = prelu_w.reshape(128, 1).astype(np.float32)

    in2 = []
    for c in range(NCORES):
        sel = core == c
        idx16, dstv = _pack_core(spid[sel], bloc[sel], dloc[sel], NL, NH)
        in2.append({"hD": hD, "hshT": hshT[c], "idx": idx16, "dstv": dstv,
                    "dinvrep": dinvrep[c], "iota": iota_np, "bias": bias_np,
                    "prelu": prw_np})
    r2 = run_bass_kernel_spmd(_nc_cache[ckey], in2,
                              core_ids=list(range(NCORES)))
    last_exec_ns.append(r2.exec_time_ns)

    out = np.empty((N, HID), dtype=np.float32)
    for c in range(NCORES):
        out[c * NSH:(c + 1) * NSH] = r2.results[c]["outT"][:, :NSH].T
    return out
